# revision 24
# baseline (speedup 1.0000x reference)
"""Trainium2 Bass kernel for nn_MultiHeadAttn (conv-QKV multi-head attention).

Sharding: data parallel over batch B=8 -> one batch item per NeuronCore.

Per-core pipeline (v2: Winograd F(2x2,3x3) convs in fp16):
  - 3x3 SAME convs computed via Winograd F(2x2,3x3): 2.25x fewer tensor-engine
    MACs than direct conv.  Input transform (B^T d B, entries 0/+-1) runs on
    the vector engine in fp16 (2x DVE mode); weights are G g G^T-transformed
    on the host and streamed as fp16; the 16 pointwise GEMMs accumulate in
    PSUM fp32.  Output transform (A^T Y A) runs on DVE with the conv bias
    folded into fused scalar_tensor_tensor ops.
  - Internally pixels use a deinterleaved order f' = (y, x%2, x//2) so every
    transform op reads/writes packed fp16 (2x DVE); Wo's columns are permuted
    on the host to match, heads stay intact since head = y//2.
  - Q^T/K^T produced by PE transposes of the conv output; V written directly
    in PV layout with a ones column (softmax denominator comes out of the PV
    matmul's 65th row).
  - Attention: S^T = K Q^T per head in fp16 (fp32 PSUM), P = exp(S/8) in
    bf16 (bf16 has the range for unshifted exp), masked by elementwise
    multiply with the 0/1 mask (fp16), PV matmul in bf16, normalization via
    reciprocal + DRAM-broadcast trick, output linear in fp16.
Host-side work: layout/padding/dtype prep plus the standard Winograd weight
transform G W G^T (a one-time 3x3->4x4 filter re-expression).
"""

import sys

if "/opt/trn_rl_repo" not in sys.path:
    sys.path.insert(0, "/opt/trn_rl_repo")

import numpy as np

_CACHE = {}

B = 8
C = 1024          # tokens (= conv channels)
F = 1024          # features (= H*W pixels)
NH = 16           # heads
HD = 64           # head dim


def _build_program(reps=1):
    from contextlib import ExitStack

    import concourse.bass as bass
    import concourse.mybir as mybir
    import concourse.tile as tile
    from concourse import bacc

    FP = mybir.dt.float32
    F16 = mybir.dt.float16

    nc = bacc.Bacc(None, target_bir_lowering=False)

    # Per-core inputs (host-prepped layouts)
    # x: fp16, padded 34x34, x-parity deinterleaved: [icc, ic128, 34*2*17]
    xq_d = nc.dram_tensor("xq", [8, 128, 1156], F16, kind="ExternalInput")
    xk_d = nc.dram_tensor("xk", [8, 128, 1156], F16, kind="ExternalInput")
    xv_d = nc.dram_tensor("xv", [8, 128, 1156], F16, kind="ExternalInput")
    # Winograd weights: [occ, icc, ic128, munu16 * oc128] fp16
    wq_d = nc.dram_tensor("wq", [8, 8, 128, 2048], F16, kind="ExternalInput")
    wk_d = nc.dram_tensor("wk", [8, 8, 128, 2048], F16, kind="ExternalInput")
    wv_d = nc.dram_tensor("wv", [8, 8, 128, 2048], F16, kind="ExternalInput")
    # output linear, column-permuted + transposed: [fcc, f'128, j1024] fp16
    wo_d = nc.dram_tensor("wo", [8, 128, C], F16, kind="ExternalInput")
    bq_d = nc.dram_tensor("bq", [C], FP, kind="ExternalInput")
    bk_d = nc.dram_tensor("bk", [C], FP, kind="ExternalInput")
    bv_d = nc.dram_tensor("bv", [C], FP, kind="ExternalInput")
    bo_d = nc.dram_tensor("bo", [C], FP, kind="ExternalInput")
    # mask^T as fp16 0/1: [sc, s128, t1024]
    mt_d = nc.dram_tensor("mt", [8, 128, C], F16, kind="ExternalInput")
    out_d = nc.dram_tensor("out", [C, C], FP, kind="ExternalOutput")

    with ExitStack() as ctx:
        tc = ctx.enter_context(tile.TileContext(nc))
        for _rep in range(reps):
            _build_body(nc, tc, bass, mybir, tile,
                        (xq_d, xk_d, xv_d, wq_d, wk_d, wv_d, wo_d,
                         bq_d, bk_d, bv_d, bo_d, mt_d, out_d))

    nc.compile()
    return nc


def _build_body(nc, tc, bass, mybir, tile, drams):
    from contextlib import ExitStack

    FP = mybir.dt.float32
    F16 = mybir.dt.float16
    BF = mybir.dt.bfloat16
    AL = mybir.AluOpType
    AF = mybir.ActivationFunctionType
    (xq_d, xk_d, xv_d, wq_d, wk_d, wv_d, wo_d,
     bq_d, bk_d, bv_d, bo_d, mt_d, out_d) = drams

    def bcast(dram_h):
        ap = dram_h[:]
        return bass.AP(tensor=ap.tensor, offset=ap.offset,
                       ap=[[0, 128]] + list(ap.ap))

    def sub_ap(t_ap, off, dims):
        """Strided free-dim view: keep partition dim, custom free dims."""
        return bass.AP(tensor=t_ap.tensor, offset=t_ap.offset + off,
                       ap=[list(t_ap.ap)[0]] + [list(d) for d in dims])

    with ExitStack() as ctx:
        persist = ctx.enter_context(tc.tile_pool(name="persist", bufs=1))
        qt = persist.tile([128, 8, C], F16)        # Q^T: [f'%128, f'//128, t]
        kt = persist.tile([128, 8, C], F16)        # K^T
        vt = persist.tile([128, 8, NH, HD + 1], BF)  # V: [s%128, s//128, h, d'+1]
        bqp = persist.tile([128, 8], FP)
        bkp = persist.tile([128, 8], FP)
        bvp = persist.tile([128, 8], FP)
        nc.gpsimd.dma_start(out=bqp, in_=bq_d[:].rearrange("(a p) -> p a", p=128))
        nc.gpsimd.dma_start(out=bkp, in_=bk_d[:].rearrange("(a p) -> p a", p=128))
        nc.gpsimd.dma_start(out=bvp, in_=bv_d[:].rearrange("(a p) -> p a", p=128))
        from concourse.masks import make_identity
        ident = persist.tile([128, 128], FP)
        make_identity(nc, ident)
        ident16 = persist.tile([128, 128], F16)
        nc.vector.tensor_copy(out=ident16, in_=ident)
        # ones column for the softmax denominator
        nc.vector.memset(vt[:, :, :, HD:HD + 1], 1.0)

        # ---------------- conv phase (Winograd F(2x2,3x3)) ----------------
        with tc.tile_pool(name="xtp", bufs=1) as xtp, \
                tc.tile_pool(name="wtp", bufs=4) as wtp, \
                tc.tile_pool(name="xpp", bufs=3) as xpp, \
                tc.tile_pool(name="x1p", bufs=2) as x1p, \
                tc.tile_pool(name="ysp", bufs=2) as ysp, \
                tc.tile_pool(name="ttp", bufs=2) as ttp, \
                tc.tile_pool(name="scp", bufs=2) as scp, \
                tc.tile_pool(name="stp", bufs=4) as stp, \
                tc.tile_pool(name="psc", bufs=3, space="PSUM") as psc, \
                tc.tile_pool(name="psT", bufs=2, space="PSUM") as psT:

            # x-tilde: per icc [ic128, munu16, tile256] fp16
            xt = [xtp.tile([128, 16, 256], F16, name=f"xt{i}") for i in range(8)]

            def input_transform(xd, icc):
                xp = xpp.tile([128, 34, 2, 17], F16, tag="xp", name="xp")
                nc.sync.dma_start(out=xp, in_=xd[icc])
                x1 = x1p.tile([128, 4, 16, 34], F16, tag="x1", name="x1")

                def row(r):  # [128, ty16, e2, xh17] rows 2ty+r of padded x
                    return xp[:, r:r + 31:2]

                def x1m(m):  # [128, ty16, e2, xh17] view of x1[:, m]
                    return x1[:, m].rearrange("p a (e x) -> p a e x", x=17)

                # stage 1: B^T on rows
                nc.vector.tensor_sub(x1m(0), row(0), row(2))
                nc.vector.tensor_add(x1m(1), row(1), row(2))
                nc.vector.tensor_sub(x1m(2), row(2), row(1))
                nc.vector.tensor_sub(x1m(3), row(1), row(3))
                # stage 2: B^T on cols (deinterleaved: col 2tx+s)
                x1v = x1.rearrange("p m t (e x) -> p m t e x", x=17)
                c0 = x1v[:, :, :, 0, 0:16]   # [128, 4mu, 16ty, 16tx]
                c1 = x1v[:, :, :, 1, 0:16]
                c2 = x1v[:, :, :, 0, 1:17]
                c3 = x1v[:, :, :, 1, 1:17]
                xtv = xt[icc].rearrange("p (m n) t -> p m n t", n=4)

                def dv(n):  # [128, 4mu, 256tiles]
                    return xtv[:, :, n]

                nc.vector.tensor_sub(dv(0), c0, c2)
                nc.vector.tensor_add(dv(1), c1, c2)
                nc.vector.tensor_sub(dv(2), c2, c1)
                nc.vector.tensor_sub(dv(3), c1, c3)

            def out_transform(ps, bpp, occ, tb, kind, dst):
                """PSUM [128, 16munu, 64] -> A^T Y A + bias -> dst (fp16)."""
                ys = ysp.tile([128, 16, 64], F16, tag="ys", name="ys")
                nc.scalar.copy(out=ys, in_=ps)
                ysv = ys.rearrange("p (m n) t -> p m n t", n=4)
                tt = ttp.tile([128, 2, 4, 64], F16, tag="tt", name="tt")
                # stage 1: A^T on mu  (mu'0 = y0+y1+y2, mu'1 = y1-y2-y3)
                nc.vector.tensor_add(tt[:, 0], ysv[:, 0], ysv[:, 1])
                nc.vector.tensor_add(tt[:, 0], tt[:, 0], ysv[:, 2])
                nc.vector.tensor_sub(tt[:, 1], ysv[:, 1], ysv[:, 2])
                nc.vector.tensor_sub(tt[:, 1], tt[:, 1], ysv[:, 3])
                # stage 2: A^T on nu + bias
                sc = scp.tile([128, 2, 64], F16, tag="sc", name="sc")
                bias = bpp[:, occ:occ + 1]
                if kind == "v":
                    # dst = vt; h = 4tb+ty, d' = mu'*32 + nu'*16 + tx
                    base = occ * (NH * 65) + (4 * tb) * 65
                    step = 65
                else:
                    # dst = stash quarter [128, 256]; local y = 2ty+mu'
                    base = 0
                    step = 64

                def dv(mp, nup):  # [128, ty4, tx16] dst view (3D for neuronxcc)
                    return sub_ap(dst[:], base + mp * 32 + nup * 16,
                                  [[step, 4], [1, 16]])

                nc.vector.tensor_add(sc, tt[:, :, 0], tt[:, :, 1])
                for mp in range(2):
                    nc.vector.scalar_tensor_tensor(
                        out=dv(mp, 0), in0=sc[:, mp], scalar=bias,
                        in1=tt[:, mp, 2], op0=AL.add, op1=AL.add)
                nc.vector.tensor_sub(sc, tt[:, :, 1], tt[:, :, 2])
                for mp in range(2):
                    nc.vector.scalar_tensor_tensor(
                        out=dv(mp, 1), in0=sc[:, mp], scalar=bias,
                        in1=tt[:, mp, 3], op0=AL.add, op1=AL.subtract)

            def load_w(wd, occ):
                # W-tile halves on two independent DMA rings (sync HWDGE
                # + gpsimd SWDGE) so weight streaming keeps up with PE
                wt = []
                for ih, eng in ((0, nc.sync), (1, nc.gpsimd)):
                    w = wtp.tile([128, 4, 16, 128], F16, tag="wt", name="wt")
                    eng.dma_start(
                        out=w,
                        in_=wd[occ][ih * 4:(ih + 1) * 4].transpose([1, 0, 2]))
                    wt.append(w)
                return wt

            def do_conv(xd, wd, bpp, kind, dstq=None):
                wt0 = load_w(wd, 0)
                for icc in range(8):
                    input_transform(xd, icc)
                for occ in range(8):
                    wt = wt0 if occ == 0 else load_w(wd, occ)
                    for tb in range(4):
                        if kind == "qk":
                            st = stp.tile([128, 256], F16, tag="st", name="st")
                        ps = psc.tile([128, 16, 64], FP, tag="ps", name="ps")
                        # ps is 4KB = two 2KB psum zero regions (mn 0-7, 8-15):
                        # open/close each region's accumulation group once.
                        for icc in range(8):
                            w_ap = wt[icc // 4][:, icc % 4]
                            x_ap = xt[icc][:, :, tb * 64:(tb + 1) * 64]
                            for mn in range(16):
                                nc.tensor.matmul(
                                    ps[:, mn], w_ap[:, mn], x_ap[:, mn],
                                    start=(icc == 0 and mn % 8 == 0),
                                    stop=(icc == 7 and mn % 8 == 7))
                        if kind == "v":
                            out_transform(ps, bpp, occ, tb, "v", vt)
                        else:
                            out_transform(ps, bpp, occ, tb, "qk", st)
                            for half in (0, 1):
                                fcl = 2 * tb + half
                                # padded to a full 2KB psum zero region
                                pt = psT.tile([128, 1024], F16, tag="pt",
                                              name="pt")
                                nc.tensor.transpose(
                                    pt[:, 0:128],
                                    st[:, half * 128:(half + 1) * 128],
                                    ident16)
                                nc.vector.tensor_copy(
                                    out=dstq[:, fcl, occ * 128:(occ + 1) * 128],
                                    in_=pt[:, 0:128])
            do_conv(xq_d, wq_d, bqp, "qk", dstq=qt)
            do_conv(xk_d, wk_d, bkp, "qk", dstq=kt)
            do_conv(xv_d, wv_d, bvp, "v")

        # ---------------- attention + output linear ----------------
        with tc.tile_pool(name="attp", bufs=1) as attp, \
                tc.tile_pool(name="ptp", bufs=6) as ptp, \
                tc.tile_pool(name="oup", bufs=4) as oup, \
                tc.tile_pool(name="rbp", bufs=4) as rbp, \
                tc.tile_pool(name="dscp", bufs=4, space="DRAM") as dscp:
            mt = [attp.tile([128, C], F16, name=f"mt{i}") for i in range(8)]
            ot = [attp.tile([128, C], F16, name=f"ot{i}") for i in range(8)]
            wos = attp.tile([128, 8, C], F16)    # Wo': [f'%128, fcc, j]
            bo16 = attp.tile([1, C], F16)        # bo as a f16 row
            ones1 = attp.tile([1, 128], F16)
            for sc_ in range(8):
                nc.gpsimd.dma_start(out=mt[sc_], in_=mt_d[sc_])
            for fcc in range(8):
                nc.sync.dma_start(out=wos[:, fcc], in_=wo_d[fcc])
            bo32 = attp.tile([1, C], FP)
            nc.gpsimd.dma_start(out=bo32, in_=bo_d[:].rearrange(
                "(a b) -> a b", a=1))
            nc.vector.tensor_copy(out=bo16, in_=bo32)
            nc.vector.memset(ones1, 1.0)

            with tc.tile_pool(name="stg", bufs=2) as stg, \
                    tc.tile_pool(name="psS", bufs=2, space="PSUM") as psS, \
                    tc.tile_pool(name="psO", bufs=2, space="PSUM") as psO:
                def norm_steps(fc, ous):
                    # deferred + spread through the next fc's loop so the
                    # DVE/Pool queues never head-of-line block the PV chain
                    rbss = {}
                    for hh, ou in ous:
                        yield lambda hh=hh, ou=ou: nc.vector.reciprocal(
                            out=ou[64:65, :], in_=ou[64:65, :])
                    for hh, ou in ous:
                        def _bc(hh=hh, ou=ou):
                            eng = nc.gpsimd if hh == 0 else nc.sync
                            dsc = dscp.tile([1, C], FP, tag="d", name="dsc")
                            eng.dma_start(out=dsc, in_=ou[64:65, :])
                            rbs = rbp.tile([64, C], FP, tag="rbs", name="rbs")
                            dap = dsc[0:1, :]
                            eng.dma_start(out=rbs, in_=bass.AP(
                                tensor=dap.tensor, offset=dap.offset,
                                ap=[[0, 64]] + list(dap.ap)[1:]))
                            rbss[hh] = rbs
                        yield _bc
                    for hh, ou in ous:
                        yield lambda hh=hh, ou=ou: nc.vector.tensor_mul(
                            ot[fc][64 * hh:64 * hh + 64, :], ou[0:64, :],
                            rbss[hh])

                pending = None
                for fc in range(8):
                    po = {}
                    for hh in (0, 1):
                        po[hh] = psO.tile([65, C], FP, tag="po", name=f"po{hh}")
                    for tkc in range(8):
                        if pending is not None and 1 <= tkc <= 6:
                            step = next(pending, None)
                            if step is not None:
                                step()
                        for hh in (0, 1):
                            pb = 64 * hh
                            sps = psS.tile([128, C], FP, tag="s", name="sps")
                            for qh in (0, 1):
                                nc.tensor.matmul(
                                    sps[:, qh * 512:(qh + 1) * 512],
                                    kt[pb:pb + 64, fc,
                                       tkc * 128:(tkc + 1) * 128],
                                    qt[pb:pb + 64, fc,
                                       qh * 512:(qh + 1) * 512],
                                    start=True, stop=True)
                            ptt = ptp.tile([128, C], BF, tag="pt", name="ptt")
                            nc.scalar.activation(
                                out=ptt, in_=sps, func=AF.Exp, scale=0.125)
                            ptm = ptp.tile([128, C], BF, tag="pt", name="ptm")
                            nc.vector.tensor_mul(ptm, ptt, mt[tkc])
                            for qh in (0, 1):
                                nc.tensor.matmul(
                                    po[hh][:, qh * 512:(qh + 1) * 512],
                                    vt[:, tkc, 2 * fc + hh],
                                    ptm[:, qh * 512:(qh + 1) * 512],
                                    start=(tkc == 0), stop=(tkc == 7))
                            if tkc == 7 and hh == 0:
                                ou0 = oup.tile([65, C], FP, tag="ou",
                                               name="ou")
                                nc.scalar.copy(out=ou0, in_=po[0])
                    if pending is not None:
                        for step in pending:
                            step()
                    ou1 = oup.tile([65, C], FP, tag="ou", name="ou")
                    nc.vector.tensor_copy(out=ou1, in_=po[1])
                    pending = norm_steps(fc, [(0, ou0), (1, ou1)])
                for step in pending:
                    step()

                for tcc in range(8):
                    pls = psS.tile([128, C], FP, tag="s", name="psl")
                    for jh in (0, 1):
                        # bias row broadcast into the accumulator via PE
                        nc.tensor.matmul(
                            pls[:, jh * 512:(jh + 1) * 512],
                            ones1,
                            bo16[:, jh * 512:(jh + 1) * 512],
                            start=True, stop=False)
                    for fcc in range(8):
                        lhsT = ot[fcc][:, tcc * 128:(tcc + 1) * 128]
                        for jh in (0, 1):
                            nc.tensor.matmul(
                                pls[:, jh * 512:(jh + 1) * 512],
                                lhsT,
                                wos[:, fcc, jh * 512:(jh + 1) * 512],
                                start=False, stop=(fcc == 7))
                    so = stg.tile([128, C], FP, tag="so", name="so")
                    nc.scalar.copy(out=so, in_=pls)
                    nc.sync.dma_start(
                        out=out_d[tcc * 128:(tcc + 1) * 128, :], in_=so)


# ---------------- host-side prep ----------------

_G = np.array([[1, 0, 0], [.5, .5, .5], [.5, -.5, .5], [0, 0, 1]], np.float32)


def _prep_w(W):
    """[O, I, 3, 3] -> Winograd [occ, icc, ic128, munu16*oc128] fp16."""
    W = np.asarray(W, np.float32)
    Wt = np.einsum("mr,oirs,ns->mnio", _G, W, _G).reshape(16, C, C)
    Wt = Wt.astype(np.float16)
    # [munu, i, o] -> [occ, icc, ic, munu, oc]
    Wt = Wt.reshape(16, 8, 128, 8, 128).transpose(3, 1, 2, 0, 4)
    return np.ascontiguousarray(Wt.reshape(8, 8, 128, 2048))


def _prep_x(x):
    """[C, 32, 32] -> padded fp16, x-parity deinterleaved [8, 128, 1156]."""
    xp = np.zeros((C, 34, 34), np.float32)
    xp[:, 1:33, 1:33] = x
    xp = xp.reshape(C, 34, 17, 2).transpose(0, 1, 3, 2)  # [C, 34, 2, 17]
    return np.ascontiguousarray(xp.astype(np.float16).reshape(8, 128, 1156))


def _perm():
    fp = np.arange(F)
    y = fp // 32
    r = fp % 32
    e = r // 16
    xh = r % 16
    return y * 32 + 2 * xh + e


def _prep_wo(Wo):
    """[dim, dim] -> column-permuted, transposed [fcc, f'128, j1024] fp16."""
    Wo = np.asarray(Wo, np.float32)
    wos = Wo[:, _perm()].T.astype(np.float16)
    return np.ascontiguousarray(wos.reshape(8, 128, C))


def get_program(reps=1):
    key = ("nc", reps)
    if key not in _CACHE:
        _CACHE[key] = _build_program(reps)
    return _CACHE[key]


def make_in_maps(q, k, v, Wq, bq, Wk, bk, Wv, bv, Wo, bo, mask):
    wq = _prep_w(Wq)
    wk = _prep_w(Wk)
    wv = _prep_w(Wv)
    wo = _prep_wo(Wo)
    bq, bk, bv, bo = (np.ascontiguousarray(np.asarray(b), dtype=np.float32)
                      for b in (bq, bk, bv, bo))
    in_maps = []
    for b in range(B):
        mt = np.asarray(mask[b]).T.astype(np.float16)
        in_maps.append({
            "xq": _prep_x(np.asarray(q[b]).reshape(C, 32, 32)),
            "xk": _prep_x(np.asarray(k[b]).reshape(C, 32, 32)),
            "xv": _prep_x(np.asarray(v[b]).reshape(C, 32, 32)),
            "wq": wq, "wk": wk, "wv": wv, "wo": wo,
            "bq": bq, "bk": bk, "bv": bv, "bo": bo,
            "mt": np.ascontiguousarray(mt.reshape(8, 128, C)),
        })
    return in_maps


def run(inputs, trace=False, **kw):
    from concourse.bass_utils import run_bass_kernel_spmd

    nc = get_program()
    in_maps = make_in_maps(**inputs)
    res = run_bass_kernel_spmd(nc, in_maps, list(range(B)), trace=trace, **kw)
    out = np.stack([res.results[i]["out"] for i in range(B)], axis=0)
    return out, res


def kernel(**inputs) -> np.ndarray:
    out, _ = run(inputs, trace=False)
    return out


# revision 25
# speedup vs baseline: 1.0041x; 1.0041x over previous
"""Trainium2 Bass kernel for nn_MultiHeadAttn (conv-QKV multi-head attention).

Sharding: data parallel over batch B=8 -> one batch item per NeuronCore.

Per-core pipeline (v2: Winograd F(2x2,3x3) convs in fp16):
  - 3x3 SAME convs computed via Winograd F(2x2,3x3): 2.25x fewer tensor-engine
    MACs than direct conv.  Input transform (B^T d B, entries 0/+-1) runs on
    the vector engine in fp16 (2x DVE mode); weights are G g G^T-transformed
    on the host and streamed as fp16; the 16 pointwise GEMMs accumulate in
    PSUM fp32.  Output transform (A^T Y A) runs on DVE with the conv bias
    folded into fused scalar_tensor_tensor ops.
  - Internally pixels use a deinterleaved order f' = (y, x%2, x//2) so every
    transform op reads/writes packed fp16 (2x DVE); Wo's columns are permuted
    on the host to match, heads stay intact since head = y//2.
  - Q^T/K^T produced by PE transposes of the conv output; V written directly
    in PV layout with a ones column (softmax denominator comes out of the PV
    matmul's 65th row).
  - Attention: S^T = K Q^T per head in fp16 (fp32 PSUM), P = exp(S/8) in
    bf16 (bf16 has the range for unshifted exp), masked by elementwise
    multiply with the 0/1 mask (fp16), PV matmul in bf16, normalization via
    reciprocal + DRAM-broadcast trick, output linear in fp16.
Host-side work: layout/padding/dtype prep plus the standard Winograd weight
transform G W G^T (a one-time 3x3->4x4 filter re-expression).
"""

import sys

if "/opt/trn_rl_repo" not in sys.path:
    sys.path.insert(0, "/opt/trn_rl_repo")

import numpy as np

_CACHE = {}

B = 8
C = 1024          # tokens (= conv channels)
F = 1024          # features (= H*W pixels)
NH = 16           # heads
HD = 64           # head dim


def _build_program(reps=1):
    from contextlib import ExitStack

    import concourse.bass as bass
    import concourse.mybir as mybir
    import concourse.tile as tile
    from concourse import bacc

    FP = mybir.dt.float32
    F16 = mybir.dt.float16

    nc = bacc.Bacc(None, target_bir_lowering=False)

    # Per-core inputs (host-prepped layouts)
    # x: fp16, padded 34x34, x-parity deinterleaved: [icc, ic128, 34*2*17]
    xq_d = nc.dram_tensor("xq", [8, 128, 1156], F16, kind="ExternalInput")
    xk_d = nc.dram_tensor("xk", [8, 128, 1156], F16, kind="ExternalInput")
    xv_d = nc.dram_tensor("xv", [8, 128, 1156], F16, kind="ExternalInput")
    # Winograd weights: [occ, icc, ic128, munu16 * oc128] fp16
    wq_d = nc.dram_tensor("wq", [8, 8, 128, 2048], F16, kind="ExternalInput")
    wk_d = nc.dram_tensor("wk", [8, 8, 128, 2048], F16, kind="ExternalInput")
    wv_d = nc.dram_tensor("wv", [8, 8, 128, 2048], F16, kind="ExternalInput")
    # output linear, column-permuted + transposed: [fcc, f'128, j1024] fp16
    wo_d = nc.dram_tensor("wo", [8, 128, C], F16, kind="ExternalInput")
    bq_d = nc.dram_tensor("bq", [C], FP, kind="ExternalInput")
    bk_d = nc.dram_tensor("bk", [C], FP, kind="ExternalInput")
    bv_d = nc.dram_tensor("bv", [C], FP, kind="ExternalInput")
    bo_d = nc.dram_tensor("bo", [C], FP, kind="ExternalInput")
    # mask^T as fp16 0/1: [sc, s128, t1024]
    mt_d = nc.dram_tensor("mt", [8, 128, C], F16, kind="ExternalInput")
    out_d = nc.dram_tensor("out", [C, C], FP, kind="ExternalOutput")

    with ExitStack() as ctx:
        tc = ctx.enter_context(tile.TileContext(nc))
        for _rep in range(reps):
            _build_body(nc, tc, bass, mybir, tile,
                        (xq_d, xk_d, xv_d, wq_d, wk_d, wv_d, wo_d,
                         bq_d, bk_d, bv_d, bo_d, mt_d, out_d))

    nc.compile()
    return nc


def _build_body(nc, tc, bass, mybir, tile, drams):
    from contextlib import ExitStack

    FP = mybir.dt.float32
    F16 = mybir.dt.float16
    BF = mybir.dt.bfloat16
    AL = mybir.AluOpType
    AF = mybir.ActivationFunctionType
    (xq_d, xk_d, xv_d, wq_d, wk_d, wv_d, wo_d,
     bq_d, bk_d, bv_d, bo_d, mt_d, out_d) = drams

    def bcast(dram_h):
        ap = dram_h[:]
        return bass.AP(tensor=ap.tensor, offset=ap.offset,
                       ap=[[0, 128]] + list(ap.ap))

    def sub_ap(t_ap, off, dims):
        """Strided free-dim view: keep partition dim, custom free dims."""
        return bass.AP(tensor=t_ap.tensor, offset=t_ap.offset + off,
                       ap=[list(t_ap.ap)[0]] + [list(d) for d in dims])

    with ExitStack() as ctx:
        persist = ctx.enter_context(tc.tile_pool(name="persist", bufs=1))
        qt = persist.tile([128, 8, C], F16)        # Q^T: [f'%128, f'//128, t]
        kt = persist.tile([128, 8, C], F16)        # K^T
        vt = persist.tile([128, 8, NH, HD + 1], BF)  # V: [s%128, s//128, h, d'+1]
        bqp = persist.tile([128, 8], FP)
        bkp = persist.tile([128, 8], FP)
        bvp = persist.tile([128, 8], FP)
        nc.gpsimd.dma_start(out=bqp, in_=bq_d[:].rearrange("(a p) -> p a", p=128))
        nc.gpsimd.dma_start(out=bkp, in_=bk_d[:].rearrange("(a p) -> p a", p=128))
        nc.gpsimd.dma_start(out=bvp, in_=bv_d[:].rearrange("(a p) -> p a", p=128))
        from concourse.masks import make_identity
        ident = persist.tile([128, 128], FP)
        make_identity(nc, ident)
        ident16 = persist.tile([128, 128], F16)
        nc.vector.tensor_copy(out=ident16, in_=ident)
        # ones column for the softmax denominator
        nc.vector.memset(vt[:, :, :, HD:HD + 1], 1.0)

        # ---------------- conv phase (Winograd F(2x2,3x3)) ----------------
        with tc.tile_pool(name="xtp", bufs=1) as xtp, \
                tc.tile_pool(name="wtp", bufs=4) as wtp, \
                tc.tile_pool(name="xpp", bufs=3) as xpp, \
                tc.tile_pool(name="x1p", bufs=2) as x1p, \
                tc.tile_pool(name="ysp", bufs=2) as ysp, \
                tc.tile_pool(name="ttp", bufs=2) as ttp, \
                tc.tile_pool(name="scp", bufs=2) as scp, \
                tc.tile_pool(name="stp", bufs=4) as stp, \
                tc.tile_pool(name="psc", bufs=3, space="PSUM") as psc, \
                tc.tile_pool(name="psT", bufs=2, space="PSUM") as psT:

            # x-tilde: per icc [ic128, munu16, tile256] fp16
            xt = [xtp.tile([128, 16, 256], F16, name=f"xt{i}") for i in range(8)]

            def input_transform(xd, icc):
                xp = xpp.tile([128, 34, 2, 17], F16, tag="xp", name="xp")
                nc.sync.dma_start(out=xp, in_=xd[icc])
                x1 = x1p.tile([128, 4, 16, 34], F16, tag="x1", name="x1")

                def row(r):  # [128, ty16, e2, xh17] rows 2ty+r of padded x
                    return xp[:, r:r + 31:2]

                def x1m(m):  # [128, ty16, e2, xh17] view of x1[:, m]
                    return x1[:, m].rearrange("p a (e x) -> p a e x", x=17)

                # stage 1: B^T on rows
                nc.vector.tensor_sub(x1m(0), row(0), row(2))
                nc.vector.tensor_add(x1m(1), row(1), row(2))
                nc.vector.tensor_sub(x1m(2), row(2), row(1))
                nc.vector.tensor_sub(x1m(3), row(1), row(3))
                # stage 2: B^T on cols (deinterleaved: col 2tx+s)
                x1v = x1.rearrange("p m t (e x) -> p m t e x", x=17)
                c0 = x1v[:, :, :, 0, 0:16]   # [128, 4mu, 16ty, 16tx]
                c1 = x1v[:, :, :, 1, 0:16]
                c2 = x1v[:, :, :, 0, 1:17]
                c3 = x1v[:, :, :, 1, 1:17]
                xtv = xt[icc].rearrange("p (m n) t -> p m n t", n=4)

                def dv(n):  # [128, 4mu, 256tiles]
                    return xtv[:, :, n]

                nc.vector.tensor_sub(dv(0), c0, c2)
                nc.vector.tensor_add(dv(1), c1, c2)
                nc.vector.tensor_sub(dv(2), c2, c1)
                nc.vector.tensor_sub(dv(3), c1, c3)

            def out_transform(ps, bpp, occ, tb, kind, dst):
                """PSUM [128, 16munu, 64] -> A^T Y A + bias -> dst (fp16)."""
                ys = ysp.tile([128, 16, 64], F16, tag="ys", name="ys")
                nc.scalar.copy(out=ys, in_=ps)
                ysv = ys.rearrange("p (m n) t -> p m n t", n=4)
                tt = ttp.tile([128, 2, 4, 64], F16, tag="tt", name="tt")
                # stage 1: A^T on mu  (mu'0 = y0+y1+y2, mu'1 = y1-y2-y3)
                nc.vector.tensor_add(tt[:, 0], ysv[:, 0], ysv[:, 1])
                nc.vector.tensor_add(tt[:, 0], tt[:, 0], ysv[:, 2])
                nc.vector.tensor_sub(tt[:, 1], ysv[:, 1], ysv[:, 2])
                nc.vector.tensor_sub(tt[:, 1], tt[:, 1], ysv[:, 3])
                # stage 2: A^T on nu + bias
                sc = scp.tile([128, 2, 64], F16, tag="sc", name="sc")
                bias = bpp[:, occ:occ + 1]
                if kind == "v":
                    # dst = vt; h = 4tb+ty, d' = mu'*32 + nu'*16 + tx
                    base = occ * (NH * 65) + (4 * tb) * 65
                    step = 65
                else:
                    # dst = stash quarter [128, 256]; local y = 2ty+mu'
                    base = 0
                    step = 64

                def dv(mp, nup):  # [128, ty4, tx16] dst view (3D for neuronxcc)
                    return sub_ap(dst[:], base + mp * 32 + nup * 16,
                                  [[step, 4], [1, 16]])

                nc.vector.tensor_add(sc, tt[:, :, 0], tt[:, :, 1])
                for mp in range(2):
                    nc.vector.scalar_tensor_tensor(
                        out=dv(mp, 0), in0=sc[:, mp], scalar=bias,
                        in1=tt[:, mp, 2], op0=AL.add, op1=AL.add)
                nc.vector.tensor_sub(sc, tt[:, :, 1], tt[:, :, 2])
                for mp in range(2):
                    nc.vector.scalar_tensor_tensor(
                        out=dv(mp, 1), in0=sc[:, mp], scalar=bias,
                        in1=tt[:, mp, 3], op0=AL.add, op1=AL.subtract)

            def load_w(wd, occ):
                # W-tile halves on two independent DMA rings (sync HWDGE
                # + gpsimd SWDGE) so weight streaming keeps up with PE
                wt = []
                for ih, eng in ((0, nc.sync), (1, nc.gpsimd)):
                    w = wtp.tile([128, 4, 16, 128], F16, tag="wt", name="wt")
                    eng.dma_start(
                        out=w,
                        in_=wd[occ][ih * 4:(ih + 1) * 4].transpose([1, 0, 2]))
                    wt.append(w)
                return wt

            def do_conv(xd, wd, bpp, kind, dstq=None):
                wt0 = load_w(wd, 0)
                for icc in range(8):
                    input_transform(xd, icc)
                for occ in range(8):
                    wt = wt0 if occ == 0 else load_w(wd, occ)
                    for tb in range(4):
                        if kind == "qk":
                            st = stp.tile([128, 256], F16, tag="st", name="st")
                        ps = psc.tile([128, 16, 64], FP, tag="ps", name="ps")
                        # ps is 4KB = two 2KB psum zero regions (mn 0-7, 8-15):
                        # open/close each region's accumulation group once.
                        for icc in range(8):
                            w_ap = wt[icc // 4][:, icc % 4]
                            x_ap = xt[icc][:, :, tb * 64:(tb + 1) * 64]
                            for mn in range(16):
                                nc.tensor.matmul(
                                    ps[:, mn], w_ap[:, mn], x_ap[:, mn],
                                    start=(icc == 0 and mn % 8 == 0),
                                    stop=(icc == 7 and mn % 8 == 7))
                        if kind == "v":
                            out_transform(ps, bpp, occ, tb, "v", vt)
                        else:
                            out_transform(ps, bpp, occ, tb, "qk", st)
                            for half in (0, 1):
                                fcl = 2 * tb + half
                                # padded to a full 2KB psum zero region
                                pt = psT.tile([128, 1024], F16, tag="pt",
                                              name="pt")
                                nc.tensor.transpose(
                                    pt[:, 0:128],
                                    st[:, half * 128:(half + 1) * 128],
                                    ident16)
                                nc.vector.tensor_copy(
                                    out=dstq[:, fcl, occ * 128:(occ + 1) * 128],
                                    in_=pt[:, 0:128])
            do_conv(xq_d, wq_d, bqp, "qk", dstq=qt)
            do_conv(xk_d, wk_d, bkp, "qk", dstq=kt)
            do_conv(xv_d, wv_d, bvp, "v")

        # ---------------- attention + output linear ----------------
        with tc.tile_pool(name="attp", bufs=1) as attp, \
                tc.tile_pool(name="ptp", bufs=6) as ptp, \
                tc.tile_pool(name="oup", bufs=4) as oup, \
                tc.tile_pool(name="rbp", bufs=4) as rbp, \
                tc.tile_pool(name="dscp", bufs=4, space="DRAM") as dscp:
            mt = [attp.tile([128, C], F16, name=f"mt{i}") for i in range(8)]
            ot = [attp.tile([128, C], F16, name=f"ot{i}") for i in range(8)]
            wos = attp.tile([128, 8, C], F16)    # Wo': [f'%128, fcc, j]
            bob = attp.tile([128, C], FP)
            for sc_ in range(8):
                nc.gpsimd.dma_start(out=mt[sc_], in_=mt_d[sc_])
            for fcc in range(8):
                nc.sync.dma_start(out=wos[:, fcc], in_=wo_d[fcc])
            nc.gpsimd.dma_start(out=bob, in_=bcast(bo_d))

            with tc.tile_pool(name="stg", bufs=2) as stg, \
                    tc.tile_pool(name="psS", bufs=2, space="PSUM") as psS, \
                    tc.tile_pool(name="psO", bufs=2, space="PSUM") as psO:
                def norm_steps(fc, ous):
                    # deferred + spread through the next fc's loop so the
                    # DVE/Pool queues never head-of-line block the PV chain
                    rbss = {}
                    for hh, ou in ous:
                        yield lambda hh=hh, ou=ou: nc.vector.reciprocal(
                            out=ou[64:65, :], in_=ou[64:65, :])
                    for hh, ou in ous:
                        def _bc(hh=hh, ou=ou):
                            eng = nc.gpsimd if hh == 0 else nc.sync
                            dsc = dscp.tile([1, C], FP, tag="d", name="dsc")
                            eng.dma_start(out=dsc, in_=ou[64:65, :])
                            rbs = rbp.tile([64, C], FP, tag="rbs", name="rbs")
                            dap = dsc[0:1, :]
                            eng.dma_start(out=rbs, in_=bass.AP(
                                tensor=dap.tensor, offset=dap.offset,
                                ap=[[0, 64]] + list(dap.ap)[1:]))
                            rbss[hh] = rbs
                        yield _bc
                    for hh, ou in ous:
                        yield lambda hh=hh, ou=ou: nc.vector.tensor_mul(
                            ot[fc][64 * hh:64 * hh + 64, :], ou[0:64, :],
                            rbss[hh])

                pending = None
                for fc in range(8):
                    po = {}
                    for hh in (0, 1):
                        po[hh] = psO.tile([65, C], FP, tag="po", name=f"po{hh}")
                    for tkc in range(8):
                        if pending is not None and 1 <= tkc <= 6:
                            step = next(pending, None)
                            if step is not None:
                                step()
                        for hh in (0, 1):
                            pb = 64 * hh
                            sps = psS.tile([128, C], FP, tag="s", name="sps")
                            for qh in (0, 1):
                                nc.tensor.matmul(
                                    sps[:, qh * 512:(qh + 1) * 512],
                                    kt[pb:pb + 64, fc,
                                       tkc * 128:(tkc + 1) * 128],
                                    qt[pb:pb + 64, fc,
                                       qh * 512:(qh + 1) * 512],
                                    start=True, stop=True)
                            ptt = ptp.tile([128, C], BF, tag="pt", name="ptt")
                            nc.scalar.activation(
                                out=ptt, in_=sps, func=AF.Exp, scale=0.125)
                            ptm = ptp.tile([128, C], BF, tag="pt", name="ptm")
                            nc.vector.tensor_mul(ptm, ptt, mt[tkc])
                            for qh in (0, 1):
                                nc.tensor.matmul(
                                    po[hh][:, qh * 512:(qh + 1) * 512],
                                    vt[:, tkc, 2 * fc + hh],
                                    ptm[:, qh * 512:(qh + 1) * 512],
                                    start=(tkc == 0), stop=(tkc == 7))
                            if tkc == 7 and hh == 0:
                                ou0 = oup.tile([65, C], FP, tag="ou",
                                               name="ou")
                                nc.scalar.copy(out=ou0, in_=po[0])
                    if pending is not None:
                        for step in pending:
                            step()
                    ou1 = oup.tile([65, C], FP, tag="ou", name="ou")
                    nc.vector.tensor_copy(out=ou1, in_=po[1])
                    pending = norm_steps(fc, [(0, ou0), (1, ou1)])
                for step in pending:
                    step()

                for tcc in range(8):
                    pls = psS.tile([128, C], FP, tag="s", name="psl")
                    for fcc in range(8):
                        lhsT = ot[fcc][:, tcc * 128:(tcc + 1) * 128]
                        for jh in (0, 1):
                            nc.tensor.matmul(
                                pls[:, jh * 512:(jh + 1) * 512],
                                lhsT,
                                wos[:, fcc, jh * 512:(jh + 1) * 512],
                                start=(fcc == 0), stop=(fcc == 7))
                    so = stg.tile([128, C], FP, tag="so", name="so")
                    nc.vector.scalar_tensor_tensor(
                        out=so, in0=pls, scalar=1.0, in1=bob,
                        op0=AL.mult, op1=AL.add)
                    nc.sync.dma_start(
                        out=out_d[tcc * 128:(tcc + 1) * 128, :], in_=so)


# ---------------- host-side prep ----------------

_G = np.array([[1, 0, 0], [.5, .5, .5], [.5, -.5, .5], [0, 0, 1]], np.float32)


def _prep_w(W):
    """[O, I, 3, 3] -> Winograd [occ, icc, ic128, munu16*oc128] fp16."""
    W = np.asarray(W, np.float32)
    Wt = np.einsum("mr,oirs,ns->mnio", _G, W, _G).reshape(16, C, C)
    Wt = Wt.astype(np.float16)
    # [munu, i, o] -> [occ, icc, ic, munu, oc]
    Wt = Wt.reshape(16, 8, 128, 8, 128).transpose(3, 1, 2, 0, 4)
    return np.ascontiguousarray(Wt.reshape(8, 8, 128, 2048))


def _prep_x(x):
    """[C, 32, 32] -> padded fp16, x-parity deinterleaved [8, 128, 1156]."""
    xp = np.zeros((C, 34, 34), np.float32)
    xp[:, 1:33, 1:33] = x
    xp = xp.reshape(C, 34, 17, 2).transpose(0, 1, 3, 2)  # [C, 34, 2, 17]
    return np.ascontiguousarray(xp.astype(np.float16).reshape(8, 128, 1156))


def _perm():
    fp = np.arange(F)
    y = fp // 32
    r = fp % 32
    e = r // 16
    xh = r % 16
    return y * 32 + 2 * xh + e


def _prep_wo(Wo):
    """[dim, dim] -> column-permuted, transposed [fcc, f'128, j1024] fp16."""
    Wo = np.asarray(Wo, np.float32)
    wos = Wo[:, _perm()].T.astype(np.float16)
    return np.ascontiguousarray(wos.reshape(8, 128, C))


def get_program(reps=1):
    key = ("nc", reps)
    if key not in _CACHE:
        _CACHE[key] = _build_program(reps)
    return _CACHE[key]


def make_in_maps(q, k, v, Wq, bq, Wk, bk, Wv, bv, Wo, bo, mask):
    wq = _prep_w(Wq)
    wk = _prep_w(Wk)
    wv = _prep_w(Wv)
    wo = _prep_wo(Wo)
    bq, bk, bv, bo = (np.ascontiguousarray(np.asarray(b), dtype=np.float32)
                      for b in (bq, bk, bv, bo))
    in_maps = []
    for b in range(B):
        mt = np.asarray(mask[b]).T.astype(np.float16)
        in_maps.append({
            "xq": _prep_x(np.asarray(q[b]).reshape(C, 32, 32)),
            "xk": _prep_x(np.asarray(k[b]).reshape(C, 32, 32)),
            "xv": _prep_x(np.asarray(v[b]).reshape(C, 32, 32)),
            "wq": wq, "wk": wk, "wv": wv, "wo": wo,
            "bq": bq, "bk": bk, "bv": bv, "bo": bo,
            "mt": np.ascontiguousarray(mt.reshape(8, 128, C)),
        })
    return in_maps


def run(inputs, trace=False, **kw):
    from concourse.bass_utils import run_bass_kernel_spmd

    nc = get_program()
    in_maps = make_in_maps(**inputs)
    res = run_bass_kernel_spmd(nc, in_maps, list(range(B)), trace=trace, **kw)
    out = np.stack([res.results[i]["out"] for i in range(B)], axis=0)
    return out, res


def kernel(**inputs) -> np.ndarray:
    out, _ = run(inputs, trace=False)
    return out
